# revision 1
# baseline (speedup 1.0000x reference)
"""BsplineKAN fused kernel for Trainium2 (8 NeuronCores, batch-sharded).

Math (per reference):
  basis = truncated in-place Cox-de Boor, degree 3, K=11 uniform knots on [0,1]
  out   = LN(einsum('bik,oik->bo', basis, cp) + x @ W.T + b) * gamma + beta

Closed form used here (u = 11*x, s_m = relu(u - m)):
  basis_k (k=0..7) = (1/6) * [s_k^3 - 4 s_{k+1}^3 + 6 s_{k+2}^3 - 4 s_{k+3}^3 + s_{k+4}^3]
  basis_8  = (1/2) * [s_8^2 - 3 s_9^2 + 3 s_10^2]
  basis_9  = s_9 - 2 s_10
  basis_10 = (sign(u - 10) + 1) / 2
The linear layer is fused as a 12th basis column (feature = x, weights = W),
the +1/2 constant of basis_10 and the bias b are folded into a single K=1
ones-row matmul. Scale factors (1/6, 1/2) are folded into the control-point
matrix on the host. The big contraction (K = 12*1024) runs on the PE in bf16;
the basis is combined on-device in fp32 (the relu^3 terms reach ~1300 while
basis values are <1, so pre-combine bf16 quantization would be catastrophic).
"""

import functools
import numpy as np
import ml_dtypes

BATCH = 16384
INF = 1024
OUTF = 1024
NCORES = 8
BC = BATCH // NCORES        # 2048 batch rows per core
BMS = 512                   # batch-macro size (basis slice width)
NBM = BC // BMS             # 4 macros
IB = INF // 128             # 8 i-blocks
CPI = 12                    # feature rows per i (11 spline cols + x)
NCHUNK = IB * CPI           # 96 contraction chunks of 128
EPS = 1e-5


@functools.lru_cache(maxsize=1)
def _build_nc():
    import concourse.bass as bass
    import concourse.mybir as mybir
    import concourse.tile as tile
    from concourse import bacc

    f32 = mybir.dt.float32
    bf16 = mybir.dt.bfloat16
    AF = mybir.ActivationFunctionType
    OP = mybir.AluOpType

    nc = bacc.Bacc("TRN2", target_bir_lowering=False, debug=False)
    xT = nc.dram_tensor("xT", [INF, BC], f32, kind="ExternalInput").ap()
    cpb = nc.dram_tensor("cpb", [NCHUNK * 128, OUTF], bf16, kind="ExternalInput").ap()
    brow = nc.dram_tensor("brow", [2, OUTF], bf16, kind="ExternalInput").ap()
    gam = nc.dram_tensor("gam", [1, OUTF], f32, kind="ExternalInput").ap()
    bet = nc.dram_tensor("bet", [1, OUTF], f32, kind="ExternalInput").ap()
    out_d = nc.dram_tensor("out", [BC, OUTF], f32, kind="ExternalOutput").ap()

    with tile.TileContext(nc) as tc:
        from contextlib import ExitStack
        with ExitStack() as ctx:
            ep = ctx.enter_context
            consts = ep(tc.tile_pool(name="consts", bufs=1))
            xpool = ep(tc.tile_pool(name="xp", bufs=2))
            spool = ep(tc.tile_pool(name="sp", bufs=7))
            s2pool = ep(tc.tile_pool(name="s2p", bufs=5))
            s3pool = ep(tc.tile_pool(name="s3p", bufs=8))
            tpool = ep(tc.tile_pool(name="tp", bufs=5))
            bpool = ep(tc.tile_pool(name="bp", bufs=3))
            wpool = ep(tc.tile_pool(name="wp", bufs=4))
            zpool = ep(tc.tile_pool(name="zp", bufs=2))
            stpool = ep(tc.tile_pool(name="stp", bufs=2))
            ypool = ep(tc.tile_pool(name="yp", bufs=2))
            ppool = ep(tc.tile_pool(name="pp", bufs=8, space="PSUM"))

            gamma_t = consts.tile([128, OUTF], f32)
            nc.sync.dma_start(out=gamma_t, in_=gam.partition_broadcast(128))
            beta_t = consts.tile([128, OUTF], f32)
            nc.sync.dma_start(out=beta_t, in_=bet.partition_broadcast(128))
            brow_t = consts.tile([2, OUTF], bf16)
            nc.sync.dma_start(out=brow_t, in_=brow)
            ones_t = consts.tile([2, 128], bf16)
            nc.vector.memset(ones_t, 1.0)
            # col 0: eps for LN; cols 1..11: -m ACT bias constants
            mconst = consts.tile([128, 12], f32)
            nc.vector.memset(mconst[:, 0:1], EPS)
            for m in range(11):
                nc.vector.memset(mconst[:, m + 1:m + 2], -float(m))

            for bm in range(NBM):
                psums = [[ppool.tile([128, 512], f32, name="psum", tag="psum")
                          for _ in range(2)]
                         for _ in range(4)]
                for ib in range(IB):
                    xt = xpool.tile([128, BMS], f32)
                    nc.sync.dma_start(
                        out=xt, in_=xT[ib * 128:(ib + 1) * 128,
                                       bm * BMS:(bm + 1) * BMS])
                    # rotating tiles: s_m = relu(11x-m); s2 = s^2 (ACT);
                    # s3 = s^2 * s (GPSIMD)
                    s_l, s2_l, s3_l = [], [], []
                    for m in range(11):
                        sm = spool.tile([128, BMS], f32, name="sm", tag="sm")
                        nc.scalar.activation(out=sm, in_=xt, func=AF.Relu,
                                             bias=mconst[:, m + 1:m + 2],
                                             scale=11.0)
                        s2m = s2pool.tile([128, BMS], f32, name="s2m",
                                          tag="s2m")
                        nc.scalar.activation(out=s2m, in_=sm, func=AF.Square)
                        s3m = s3pool.tile([128, BMS], f32, name="s3m",
                                          tag="s3m")
                        nc.gpsimd.tensor_mul(s3m, s2m, sm)
                        s_l.append(sm)
                        s2_l.append(s2m)
                        s3_l.append(s3m)

                    bsl = bpool.tile([128, CPI, BMS], bf16)
                    # cubic cols: 4th difference of s^3 (1/6 folded in cpb)
                    for k in range(8):
                        t1 = tpool.tile([128, BMS], f32, name="tt", tag="tt")
                        nc.vector.scalar_tensor_tensor(
                            out=t1, in0=s3_l[k + 1], scalar=-4.0,
                            in1=s3_l[k], op0=OP.mult, op1=OP.add)
                        t2 = tpool.tile([128, BMS], f32, name="tt", tag="tt")
                        nc.vector.scalar_tensor_tensor(
                            out=t2, in0=s3_l[k + 2], scalar=6.0, in1=t1,
                            op0=OP.mult, op1=OP.add)
                        if k < 7:
                            t3 = tpool.tile([128, BMS], f32, name="tt",
                                            tag="tt")
                            nc.vector.scalar_tensor_tensor(
                                out=t3, in0=s3_l[k + 3], scalar=-4.0,
                                in1=t2, op0=OP.mult, op1=OP.add)
                            nc.vector.scalar_tensor_tensor(
                                out=bsl[:, k, :], in0=s3_l[k + 4],
                                scalar=1.0, in1=t3, op0=OP.mult, op1=OP.add)
                        else:
                            nc.vector.scalar_tensor_tensor(
                                out=bsl[:, k, :], in0=s3_l[10],
                                scalar=-4.0, in1=t2, op0=OP.mult, op1=OP.add)
                    # quadratic col 8 = s8^2 - 3 s9^2 + 3 s10^2 (1/2 folded)
                    qa = tpool.tile([128, BMS], f32, name="tt", tag="tt")
                    nc.vector.scalar_tensor_tensor(
                        out=qa, in0=s2_l[9], scalar=-3.0, in1=s2_l[8],
                        op0=OP.mult, op1=OP.add)
                    nc.vector.scalar_tensor_tensor(
                        out=bsl[:, 8, :], in0=s2_l[10], scalar=3.0, in1=qa,
                        op0=OP.mult, op1=OP.add)
                    # linear col 9 = s9 - 2 s10
                    nc.vector.scalar_tensor_tensor(
                        out=bsl[:, 9, :], in0=s_l[10], scalar=-2.0,
                        in1=s_l[9], op0=OP.mult, op1=OP.add)
                    # step col 10 as sign (affine fold in cpb + brow)
                    nc.scalar.activation(out=bsl[:, 10, :], in_=xt,
                                         func=AF.Sign,
                                         bias=mconst[:, 11:12], scale=11.0)
                    # linear-layer feature: x itself
                    nc.scalar.copy(bsl[:, 11, :], xt)

                    for c in range(CPI):
                        chunk = ib * CPI + c
                        wt = wpool.tile([128, OUTF], bf16)
                        nc.sync.dma_start(
                            out=wt,
                            in_=cpb[chunk * 128:(chunk + 1) * 128, :])
                        first = (ib == 0 and c == 0)
                        for bs_i in range(4):
                            lhsT = bsl[:, c, bs_i * 128:(bs_i + 1) * 128]
                            for oh in range(2):
                                nc.tensor.matmul(
                                    psums[bs_i][oh], lhsT,
                                    wt[:, oh * 512:(oh + 1) * 512],
                                    start=first, stop=False)

                # bias row (b + 0.5*sum_i cp[:,i,10]) via ones-row matmul
                for bs_i in range(4):
                    for oh in range(2):
                        nc.tensor.matmul(
                            psums[bs_i][oh], ones_t,
                            brow_t[:, oh * 512:(oh + 1) * 512],
                            start=False, stop=True)

                # LayerNorm epilogue
                for bs_i in range(4):
                    z = zpool.tile([128, OUTF], f32)
                    nc.scalar.copy(z[:, 0:512], psums[bs_i][0])
                    nc.scalar.copy(z[:, 512:1024], psums[bs_i][1])
                    stt = stpool.tile([128, 16], f32, name="stt2", tag="stt2")
                    stats = stt[:, 0:12].rearrange("p (g s) -> p g s", g=2)
                    mvsi = stt[:, 12:16]
                    nc.vector.bn_stats(out=stats[:, 0, :], in_=z[:, 0:512])
                    nc.vector.bn_stats(out=stats[:, 1, :], in_=z[:, 512:1024])
                    nc.vector.bn_aggr(out=mvsi[:, 0:2], in_=stats)
                    nc.scalar.activation(out=mvsi[:, 2:3], in_=mvsi[:, 1:2],
                                         func=AF.Sqrt, bias=mconst[:, 0:1])
                    nc.vector.reciprocal(out=mvsi[:, 3:4], in_=mvsi[:, 2:3])
                    y = ypool.tile([128, OUTF], f32)
                    nc.vector.tensor_scalar(
                        out=y, in0=z, scalar1=mvsi[:, 0:1],
                        scalar2=mvsi[:, 3:4],
                        op0=OP.subtract, op1=OP.mult)
                    nc.gpsimd.tensor_mul(y, y, gamma_t)
                    nc.gpsimd.tensor_add(y, y, beta_t)
                    row = bm * BMS + bs_i * 128
                    nc.sync.dma_start(out=out_d[row:row + 128, :], in_=y)

    nc.compile()
    return nc


def _host_prep(x, control_points, W, b):
    """Build per-core inputs. cpb row (k*1024+i) holds the weights for
    feature (k, i); scale factors folded in."""
    cp64 = control_points.astype(np.float64)
    blocks = []
    for k in range(12):
        if k < 8:
            blk = cp64[:, :, k].T / 6.0
        elif k == 8:
            blk = cp64[:, :, 8].T / 2.0
        elif k == 9:
            blk = cp64[:, :, 9].T
        elif k == 10:
            blk = cp64[:, :, 10].T / 2.0
        else:
            blk = W.astype(np.float64).T
        blocks.append(blk)
    # device chunk order: chunk = ib*12 + k  (i-block major, feature minor)
    kmaj = np.concatenate(blocks, axis=0).reshape(12, IB, 128, OUTF)
    cpb = np.ascontiguousarray(
        kmaj.transpose(1, 0, 2, 3).reshape(12 * INF, OUTF)
    ).astype(ml_dtypes.bfloat16)
    brow_f64 = b.astype(np.float64) + 0.5 * cp64[:, :, 10].sum(axis=1)
    brow_hi = brow_f64.astype(ml_dtypes.bfloat16)
    brow_lo = (brow_f64 - brow_hi.astype(np.float64)).astype(ml_dtypes.bfloat16)
    brow = np.ascontiguousarray(np.stack([brow_hi, brow_lo], axis=0))
    xT = np.ascontiguousarray(x.T)  # [INF, BATCH]
    return xT, cpb, brow


def kernel(x, control_points, W, b, gamma, beta):
    from concourse.bass_utils import run_bass_kernel_spmd

    xT, cpb, brow = _host_prep(x, control_points, W, b)
    gam = np.ascontiguousarray(gamma.astype(np.float32))[None, :]
    bet = np.ascontiguousarray(beta.astype(np.float32))[None, :]

    nc = _build_nc()
    in_maps = []
    for c in range(NCORES):
        in_maps.append({
            "xT": np.ascontiguousarray(xT[:, c * BC:(c + 1) * BC]),
            "cpb": cpb,
            "brow": brow,
            "gam": gam,
            "bet": bet,
        })
    res = run_bass_kernel_spmd(nc, in_maps, list(range(NCORES)))
    out = np.concatenate([res.results[c]["out"] for c in range(NCORES)], axis=0)
    return out



# revision 63
# speedup vs baseline: 1.1070x; 1.1070x over previous
"""BsplineKAN fused kernel for Trainium2 (8 NeuronCores, batch-sharded).

Math (per reference):
  basis = truncated in-place Cox-de Boor, degree 3, K=11 uniform knots on [0,1]
  out   = LN(einsum('bik,oik->bo', basis, cp) + x @ W.T + b) * gamma + beta

Closed form used here (u = 11*x, s_m = relu(u - m)):
  basis_k (k=0..7) = (1/6) * [s_k^3 - 4 s_{k+1}^3 + 6 s_{k+2}^3 - 4 s_{k+3}^3 + s_{k+4}^3]
  basis_8  = (1/2) * [s_8^2 - 3 s_9^2 + 3 s_10^2]
  basis_9  = s_9 - 2 s_10
  basis_10 = (sign(u - 10) + 1) / 2
The linear layer is fused as a 12th basis column (feature = x, weights = W),
the +1/2 constant of basis_10 and the bias b are folded into a single K=2
ones-row matmul (bf16 hi+lo rows). Scale factors (1/6, 1/2) are folded into
the control-point matrix on the host. The big contraction (K = 12*1024) runs
on the PE in bf16; the basis is combined on-device in fp32.

Schedule: the basis for pipeline stage t+1 (one (macro, i-block) step) is
emitted ahead of stage t's matmuls so ACT/DVE/Pool run a full stage ahead of
the PE. The cubic columns use a pair-sharing 4th-difference form
  col_k = (s3[k] + s3[k+4]) + 6*s3[k+2] - 4*(s3[k+1] + s3[k+3])
so the adds land on DVE and the fused scalar-tensor-tensor ops on Pool,
balancing both engines under the PE's ~662us of matmul work. Output DMAs are
emitted one stage late (their data is long since ready) so they never block
the weight-prefetch DMA queue, and the per-batch-block bias matmuls are
interleaved with the last chunk so PSUM banks free up staggered.
"""

import functools
import numpy as np
import ml_dtypes

BATCH = 16384
INF = 1024
OUTF = 1024
NCORES = 8
BC = BATCH // NCORES        # 2048 batch rows per core
BMS = 512                   # batch-macro size (basis slice width)
NBM = BC // BMS             # 4 macros
IB = INF // 128             # 8 i-blocks
CPI = 12                    # feature rows per i (11 spline cols + x)
NCHUNK = IB * CPI           # 96 contraction chunks of 128
EPS = 1e-5
CORDER = [11, 10, 9, 8, 0, 1, 2, 3, 4, 5, 6, 7]  # chunk emission order


@functools.lru_cache(maxsize=4)
def _build_nc(apply_gamma=True, apply_beta=True):
    import concourse.bass as bass
    import concourse.mybir as mybir
    import concourse.tile as tile
    from concourse import bacc

    f32 = mybir.dt.float32
    bf16 = mybir.dt.bfloat16
    AF = mybir.ActivationFunctionType
    OP = mybir.AluOpType

    nc = bacc.Bacc("TRN2", target_bir_lowering=False, debug=False)
    xT = nc.dram_tensor("xT", [INF, BC], f32, kind="ExternalInput").ap()
    cpb = nc.dram_tensor("cpb", [NCHUNK * 128, OUTF], bf16, kind="ExternalInput").ap()
    brow = nc.dram_tensor("brow", [2, OUTF], bf16, kind="ExternalInput").ap()
    gam = nc.dram_tensor("gam", [1, OUTF], f32, kind="ExternalInput").ap()
    bet = nc.dram_tensor("bet", [1, OUTF], f32, kind="ExternalInput").ap()
    out_d = nc.dram_tensor("out", [BC, OUTF], f32, kind="ExternalOutput").ap()

    with tile.TileContext(nc) as tc:
        from contextlib import ExitStack
        with ExitStack() as ctx:
            ep = ctx.enter_context
            consts = ep(tc.tile_pool(name="consts", bufs=1))
            xpool = ep(tc.tile_pool(name="xp", bufs=3))
            spool = ep(tc.tile_pool(name="sp", bufs=12))
            s2pool = ep(tc.tile_pool(name="s2p", bufs=12))
            s3pool = ep(tc.tile_pool(name="s3p", bufs=18))
            prpool = ep(tc.tile_pool(name="prp", bufs=7))
            capool = ep(tc.tile_pool(name="cap", bufs=2))
            colpool = ep(tc.tile_pool(name="colp", bufs=26))
            wpool = ep(tc.tile_pool(name="wp", bufs=14))
            ypool = ep(tc.tile_pool(name="yp", bufs=5))
            sqpool = ep(tc.tile_pool(name="sqp", bufs=2))
            stpool = ep(tc.tile_pool(name="stp", bufs=9))
            ppool = ep(tc.tile_pool(name="pp", bufs=8, space="PSUM"))

            # --- constants (emitted first; x/weight DMAs overtake via queue) ---
            mconst = consts.tile([128, 16], f32)
            nc.vector.memset(mconst[:, 0:1], EPS)
            # warmup ASAP: the ACT function-table load (~1.3us) overlaps the
            # first x-tile DMA instead of delaying the first basis column
            nc.scalar.copy(mconst[:, 12:13], mconst[:, 0:1])
            for m in range(11):
                nc.vector.memset(mconst[:, m + 1:m + 2], -float(m))
            ones_t = consts.tile([2, 128], bf16)
            nc.vector.memset(ones_t, 1.0)
            brow_t = consts.tile([2, OUTF], bf16)
            gamma_t = (consts.tile([128, OUTF], f32, name="gamma_t")
                       if apply_gamma else None)
            beta_t = (consts.tile([128, OUTF], f32, name="beta_t")
                      if apply_beta else None)

            def emit_const_dmas():
                # emitted after the first stage's weight DMAs: none of these
                # are needed before the first macro's tail
                nc.sync.dma_start(out=brow_t, in_=brow)
                if apply_gamma:
                    nc.sync.dma_start(out=gamma_t,
                                      in_=gam.partition_broadcast(128))
                if apply_beta:
                    nc.sync.dma_start(out=beta_t,
                                      in_=bet.partition_broadcast(128))

            stages = [(bm, ib) for bm in range(NBM) for ib in range(IB)]
            basis_cols = {}    # stage index -> list of 12 col tiles (bf16)
            psums = None       # current macro's psum tiles [bs][oh]
            pending_stats = []  # (bm, bs, y, stt) awaiting sum-of-squares
            pending_math = []  # (bm, bs, y, stt) awaiting mean/var math
            pending_norm = []  # (row, y, stt) awaiting normalize+gamma/beta
            pending_dma = []   # (row, y) awaiting output DMA

            def emit_basis(t, evac_bm=None):
                bm, ib = stages[t]
                xt = xpool.tile([128, BMS], f32, name="xt", tag="xt")
                nc.sync.dma_start(
                    out=xt, in_=xT[ib * 128:(ib + 1) * 128,
                                   bm * BMS:(bm + 1) * BMS])
                cols = [colpool.tile([128, BMS], bf16, name="col", tag="col")
                        for _ in range(12)]
                # immediate columns: linear-x and step(sign)
                nc.scalar.copy(cols[11], xt)
                nc.scalar.activation(out=cols[10], in_=xt, func=AF.Sign,
                                     bias=mconst[:, 11:12], scale=11.0)
                if evac_bm is not None:
                    # previous macro's PSUM evacuation, interleaved here so
                    # it lands right after the two cheap ACT columns instead
                    # of behind the whole basis stage in the engine queues
                    emit_evac(evac_bm)
                s = [None] * 11
                s2 = [None] * 11
                s3 = [None] * 11

                def relu(m):
                    s[m] = spool.tile([128, BMS], f32, name="sm", tag="sm")
                    nc.scalar.activation(out=s[m], in_=xt, func=AF.Relu,
                                         bias=mconst[:, m + 1:m + 2],
                                         scale=11.0)

                def sq(m):
                    s2[m] = s2pool.tile([128, BMS], f32, name="s2m", tag="s2m")
                    nc.scalar.activation(out=s2[m], in_=s[m], func=AF.Square)

                relu(9)
                relu(10)
                # col9 = s9 - 2*s10. DVE owns every fused scalar-tensor-
                # tensor op: the real compiler rejects TensorScalarPtr (and
                # any PSUM access) on Pool, so Pool gets plain tensor-tensor
                # ops only (the s3 muls + 6 pair-adds).
                nc.vector.scalar_tensor_tensor(
                    out=cols[9], in0=s[10], scalar=-2.0, in1=s[9],
                    op0=OP.mult, op1=OP.add)
                sq(9)
                sq(10)
                relu(8)
                sq(8)
                # col8 = (s8^2 - 3 s9^2) + 3 s10^2
                qa = capool.tile([128, BMS], f32, name="qa", tag="qa")
                nc.vector.scalar_tensor_tensor(
                    out=qa, in0=s2[9], scalar=-3.0, in1=s2[8],
                    op0=OP.mult, op1=OP.add)
                nc.vector.scalar_tensor_tensor(
                    out=cols[8], in0=s2[10], scalar=3.0, in1=qa,
                    op0=OP.mult, op1=OP.add)

                pair2 = [None] * 9  # pair2[m] = s3[m] + s3[m+2], m=1..8
                pair4 = [None] * 8  # pair4[k] = s3[k] + s3[k+4], k=0..7

                def cubic(k):
                    # colA = 6*s3[k+2] + pair4[k]; col = -4*pair2[k+1] + colA
                    ca = capool.tile([128, BMS], f32, name="ca", tag="ca")
                    nc.vector.scalar_tensor_tensor(
                        out=ca, in0=s3[k + 2], scalar=6.0, in1=pair4[k],
                        op0=OP.mult, op1=OP.add)
                    nc.vector.scalar_tensor_tensor(
                        out=cols[k], in0=pair2[k + 1], scalar=-4.0, in1=ca,
                        op0=OP.mult, op1=OP.add)

                for m in range(11):
                    if m < 8:
                        relu(m)
                        sq(m)
                    # muls live on Pool, which runs a stage ahead (s3pool is
                    # deep); pairs/cols consume s3 on DVE with that slack
                    s3[m] = s3pool.tile([128, BMS], f32, name="s3m", tag="s3m")
                    nc.gpsimd.tensor_mul(s3[m], s2[m], s[m])
                    if m == 7:
                        pair4[7] = s3[7]  # s3[11] == 0 on x in [0,1)
                    if m >= 3:
                        pair2[m - 2] = prpool.tile([128, BMS], f32,
                                                   name="p2", tag="p2")
                        peng = nc.vector if m <= 5 else nc.gpsimd
                        peng.tensor_add(pair2[m - 2], s3[m - 2], s3[m])
                    if m >= 4:
                        pair4[m - 4] = prpool.tile([128, BMS], f32,
                                                   name="p4", tag="p4")
                        nc.vector.tensor_add(pair4[m - 4], s3[m - 4], s3[m])
                        cubic(m - 4)
                # tail: pair4[7] aliases s3[7]; pair2[8] ready at m=10
                cubic(7)
                basis_cols[t] = cols

            def emit_matmuls(t):
                nonlocal psums
                bm, ib = stages[t]
                if ib == 0:
                    psums = [[ppool.tile([128, 512], f32, name="psum",
                                         tag="psum") for _ in range(2)]
                             for _ in range(4)]
                cols = basis_cols.pop(t)

                def mm(c, wt, bs_range=range(4)):
                    first = (ib == 0 and c == CORDER[0])
                    for bs_i in bs_range:
                        lhsT = cols[c][:, bs_i * 128:(bs_i + 1) * 128]
                        for oh in range(2):
                            nc.tensor.matmul(
                                psums[bs_i][oh], lhsT,
                                wt[:, oh * 512:(oh + 1) * 512],
                                start=first, stop=False)

                wts = {}
                for c in CORDER:
                    chunk = ib * CPI + c
                    wt = wpool.tile([128, OUTF], bf16, name="wt", tag="wt")
                    nc.sync.dma_start(
                        out=wt, in_=cpb[chunk * 128:(chunk + 1) * 128, :])
                    wts[c] = wt
                if ib == IB - 1:
                    # bs-major order for the last 3 chunks + bias: each
                    # psum's accumulation finishes early, so its evacuation
                    # overlaps the remaining batch-blocks' matmuls
                    for c in CORDER[:-3]:
                        mm(c, wts[c])
                    for bs_i in range(4):
                        for c in CORDER[-3:]:
                            mm(c, wts[c], bs_range=(bs_i,))
                        for oh in range(2):
                            nc.tensor.matmul(
                                psums[bs_i][oh], ones_t,
                                brow_t[:, oh * 512:(oh + 1) * 512],
                                start=False, stop=True)
                else:
                    for c in CORDER:
                        mm(c, wts[c])

            def emit_evac(bm):
                # stt cols: 0 sum_a, 1 sum_b, 2 sq_a, 3 sq_b, 4 S, 5 Q,
                #           6 mean, 7 E[z^2], 8 mean^2, 9 var, 10 sig, 11 rstd
                # Evacuate PSUM with ACT copies (+ DVE for 2 blocks): these
                # gate the next macro's matmuls, so they get wait-ts 0 and
                # high priority — the pacing on basis stages (see emit loop)
                # keeps the scheduler from filling the boundary slots with
                # future basis work first.
                last = (bm == NBM - 1)
                with tc.tile_wait_until(0.0), tc.high_priority():
                    for bs_i in range(4):
                        stt = stpool.tile([128, 12], f32, name="stt",
                                          tag="stt")
                        y = ypool.tile([128, OUTF], f32, name="y", tag="y")
                        for oh in range(2):
                            if bs_i in (1, 2):
                                nc.vector.tensor_scalar(
                                    out=y[:, oh * 512:(oh + 1) * 512],
                                    in0=psums[bs_i][oh], scalar1=1.0,
                                    scalar2=0.0, op0=OP.mult, op1=OP.add,
                                    accum_out=stt[:, oh:oh + 1])
                            else:
                                nc.scalar.activation(
                                    out=y[:, oh * 512:(oh + 1) * 512],
                                    in_=psums[bs_i][oh], func=AF.Copy,
                                    accum_out=stt[:, oh:oh + 1])
                        pending_stats.append((bm, bs_i, y, stt))

            def squares_flush(last=False):
                # row sums + sums-of-squares off the y staging buffer,
                # deferred one stage past the evacuation (off PE's path)
                items = list(pending_stats)
                pending_stats.clear()
                for bm, bs_i, y, stt in items:
                    for oh in range(2):
                        yh = y[:, oh * 512:(oh + 1) * 512]
                        sqs = sqpool.tile([128, 512], f32, name="sqs",
                                          tag="sqs")
                        if last and bs_i in (1, 3):
                            nc.vector.scalar_tensor_tensor(
                                out=sqs, in0=yh, scalar=1.0, in1=yh,
                                op0=OP.mult, op1=OP.mult,
                                accum_out=stt[:, 2 + oh:3 + oh])
                        else:
                            nc.scalar.activation(
                                out=sqs, in_=yh, func=AF.Square,
                                accum_out=stt[:, 2 + oh:3 + oh])
                    pending_math.append((bm, bs_i, y, stt))

            def statsmath_flush():
                # per-row scalar math, a further stage later so the DVE ops
                # never head-of-line block behind the ACT squares
                items = list(pending_math)
                pending_math.clear()
                for bm, bs_i, y, stt in items:
                    nc.vector.tensor_add(stt[:, 4:5], stt[:, 0:1], stt[:, 1:2])
                    nc.vector.tensor_add(stt[:, 5:6], stt[:, 2:3], stt[:, 3:4])
                    nc.vector.tensor_scalar_mul(stt[:, 6:7], stt[:, 4:5],
                                                1.0 / OUTF)
                    nc.vector.tensor_scalar_mul(stt[:, 7:8], stt[:, 5:6],
                                                1.0 / OUTF)
                    nc.vector.scalar_tensor_tensor(
                        out=stt[:, 8:9], in0=stt[:, 6:7], scalar=1.0,
                        in1=stt[:, 6:7], op0=OP.mult, op1=OP.mult)
                    nc.vector.scalar_tensor_tensor(
                        out=stt[:, 9:10], in0=stt[:, 8:9], scalar=-1.0,
                        in1=stt[:, 7:8], op0=OP.mult, op1=OP.add)
                    nc.scalar.activation(out=stt[:, 10:11], in_=stt[:, 9:10],
                                         func=AF.Sqrt, bias=mconst[:, 0:1])
                    nc.vector.reciprocal(out=stt[:, 11:12], in_=stt[:, 10:11])
                    pending_norm.append((bm * BMS + bs_i * 128, y, stt))

            def norm_flush(limit=None):
                # normalize + gamma/beta, deferred so the big [128,1024] ops
                # stay clear of the macro boundary
                idx = 0
                n = limit if limit else len(pending_norm)
                while pending_norm and n > 0:
                    n -= 1
                    row, y, stt = pending_norm.pop(0)
                    # tensor_scalar must be DVE (Pool lacks TensorScalarPtr)
                    nc.vector.tensor_scalar(out=y, in0=y, scalar1=stt[:, 6:7],
                                            scalar2=stt[:, 11:12],
                                            op0=OP.subtract, op1=OP.mult)
                    eng = nc.gpsimd if idx % 2 == 0 else nc.vector
                    if apply_gamma:
                        eng.tensor_mul(y, y, gamma_t)
                    if apply_beta:
                        eng.tensor_add(y, y, beta_t)
                    pending_dma.append((row, y))
                    idx += 1

            def dma_flush(limit=None, dual_queue=False):
                idx = 0
                n = limit if limit else len(pending_dma)
                while pending_dma and n > 0:
                    n -= 1
                    row, y = pending_dma.pop(0)
                    eng = nc.scalar if dual_queue and idx % 2 else nc.sync
                    eng.dma_start(out=out_d[row:row + 128, :], in_=y)
                    idx += 1

            # Manual pacing: cap the scheduler's lookahead for each basis
            # stage at ~1.3 stages ahead of the PE so macro-boundary slots
            # stay open for the (wait-ts 0) PSUM evacuation ops. This only
            # constrains the compile-time instruction ORDER; runtime is
            # still fully semaphore-driven.
            STAGE_MS = 0.0207

            emit_basis(0)
            emit_matmuls(0)
            emit_const_dmas()
            for t, (bm, ib) in enumerate(stages):
                if t == 0:
                    continue
                with tc.tile_wait_until(max(0.0, (t - 1.3) * STAGE_MS)):
                    emit_basis(t, evac_bm=bm - 1 if ib == 0 else None)
                    if ib == 1:
                        squares_flush()
                    if ib == 3:
                        statsmath_flush()
                    if ib == 5:
                        norm_flush()
                    if ib == 6:
                        dma_flush()
                emit_matmuls(t)
            emit_evac(NBM - 1)
            squares_flush(last=True)
            statsmath_flush()
            norm_flush()
            dma_flush(dual_queue=True)

    nc.compile()
    return nc


def _host_prep(x, control_points, W, b):
    """Build per-core inputs. cpb row (ib*12+k)*128+r holds the weights for
    feature (k, i=ib*128+r); scale factors folded in."""
    cp64 = control_points.astype(np.float64)
    blocks = []
    for k in range(12):
        if k < 8:
            blk = cp64[:, :, k].T / 6.0
        elif k == 8:
            blk = cp64[:, :, 8].T / 2.0
        elif k == 9:
            blk = cp64[:, :, 9].T
        elif k == 10:
            blk = cp64[:, :, 10].T / 2.0
        else:
            blk = W.astype(np.float64).T
        blocks.append(blk)
    # device chunk order: chunk = ib*12 + k  (i-block major, feature minor)
    kmaj = np.concatenate(blocks, axis=0).reshape(12, IB, 128, OUTF)
    cpb = np.ascontiguousarray(
        kmaj.transpose(1, 0, 2, 3).reshape(12 * INF, OUTF)
    ).astype(ml_dtypes.bfloat16)
    brow_f64 = b.astype(np.float64) + 0.5 * cp64[:, :, 10].sum(axis=1)
    brow_hi = brow_f64.astype(ml_dtypes.bfloat16)
    brow_lo = (brow_f64 - brow_hi.astype(np.float64)).astype(ml_dtypes.bfloat16)
    brow = np.ascontiguousarray(np.stack([brow_hi, brow_lo], axis=0))
    xT = np.ascontiguousarray(x.T)  # [INF, BATCH]
    return xT, cpb, brow


_last_flags = (True, True)


def kernel(x, control_points, W, b, gamma, beta):
    global _last_flags
    from concourse.bass_utils import run_bass_kernel_spmd

    xT, cpb, brow = _host_prep(x, control_points, W, b)
    gam = np.ascontiguousarray(gamma.astype(np.float32))[None, :]
    bet = np.ascontiguousarray(beta.astype(np.float32))[None, :]

    # multiplying by an all-ones gamma / adding an all-zeros beta is the
    # identity; build a variant without those ops when the inputs allow
    _last_flags = (bool(np.any(gamma != 1.0)), bool(np.any(beta != 0.0)))
    nc = _build_nc(*_last_flags)
    in_maps = []
    for c in range(NCORES):
        in_maps.append({
            "xT": np.ascontiguousarray(xT[:, c * BC:(c + 1) * BC]),
            "cpb": cpb,
            "brow": brow,
            "gam": gam,
            "bet": bet,
        })
    res = run_bass_kernel_spmd(nc, in_maps, list(range(NCORES)))
    out = np.concatenate([res.results[c]["out"] for c in range(NCORES)], axis=0)
    return out


# revision 99
# speedup vs baseline: 1.1103x; 1.0030x over previous
"""BsplineKAN fused kernel for Trainium2 (8 NeuronCores, batch-sharded).

Math (per reference):
  basis = truncated in-place Cox-de Boor, degree 3, K=11 uniform knots on [0,1]
  out   = LN(einsum('bik,oik->bo', basis, cp) + x @ W.T + b) * gamma + beta

Closed form used here (u = 11*x, s_m = relu(u - m)):
  basis_k (k=0..7) = (1/6) * [s_k^3 - 4 s_{k+1}^3 + 6 s_{k+2}^3 - 4 s_{k+3}^3 + s_{k+4}^3]
  basis_8  = (1/2) * [s_8^2 - 3 s_9^2 + 3 s_10^2]
  basis_9  = s_9 - 2 s_10
  basis_10 = (sign(u - 10) + 1) / 2
The linear layer is fused as a 12th basis column (feature = x, weights = W),
the +1/2 constant of basis_10 and the bias b are folded into a single K=2
ones-row matmul (bf16 hi+lo rows). Scale factors (1/6, 1/2) are folded into
the control-point matrix on the host. The big contraction (K = 12*1024) runs
on the PE in bf16; the basis is combined on-device in fp32.

Schedule (PE floor is ~662us of bf16 matmul; everything else hides under it):
- Pipeline unit = one (512-batch macro, 128-feature i-block) stage: ACT does
  relu/square, Pool the s3 muls + 6 pair-adds, DVE everything fused (the
  HW compiler rejects TensorScalarPtr and any PSUM access on Pool). Cubic
  columns use the pair-sharing 4th-difference form
    col_k = (s3[k] + s3[k+4]) + 6*s3[k+2] - 4*(s3[k+1] + s3[k+3]).
- Chunk order [x, sign, lin, quad, cubics] lets the PE start each stage on
  the two ACT-only columns.
- The tile scheduler orders each engine's stream from its own lookahead
  simulation, so every stage's basis ops carry a manual wait-timestamp
  (~1.3 stages behind the PE's pace): that caps engine run-ahead and keeps
  macro-boundary slots free for the PSUM-evacuating copies (wait-ts 0,
  high priority), which are the only thing the next macro's matmuls wait
  on. The last 3 chunks of each macro run batch-block-major so the four
  PSUM pairs complete (and evacuate) staggered.
- LayerNorm runs as a software pipeline spread over the following macro:
  evacuation copies with accumulated row-sums (ACT/DVE) at the boundary,
  sums-of-squares (ACT) at i-block 1, tiny per-row mean/var math (DVE) at
  i-block 3, normalize (+gamma/beta when not identity - they are identity
  for this model's inputs, detected on host) at i-block 5, output DMA at
  i-block 6. Each cross-engine hop is a stage apart, so no engine ever
  head-of-line blocks on another. The final macro instead normalizes
  straight out of PSUM (bn_stats + ACT Identity with per-row scale/bias,
  split across ACT/DVE) with half-tile output DMAs on both HWDGE queues.
"""

import functools
import numpy as np
import ml_dtypes

BATCH = 16384
INF = 1024
OUTF = 1024
NCORES = 8
BC = BATCH // NCORES        # 2048 batch rows per core
BMS = 512                   # batch-macro size (basis slice width)
NBM = BC // BMS             # 4 macros
IB = INF // 128             # 8 i-blocks
CPI = 12                    # feature rows per i (11 spline cols + x)
NCHUNK = IB * CPI           # 96 contraction chunks of 128
EPS = 1e-5
CORDER = [11, 10, 9, 0, 1, 2, 3, 4, 5, 6, 7, 8]  # chunk emission order


@functools.lru_cache(maxsize=4)
def _build_nc(apply_gamma=True, apply_beta=True):
    import concourse.bass as bass
    import concourse.mybir as mybir
    import concourse.tile as tile
    from concourse import bacc

    f32 = mybir.dt.float32
    bf16 = mybir.dt.bfloat16
    AF = mybir.ActivationFunctionType
    OP = mybir.AluOpType

    nc = bacc.Bacc("TRN2", target_bir_lowering=False, debug=False)
    xT = nc.dram_tensor("xT", [INF, BC], f32, kind="ExternalInput").ap()
    cpb = nc.dram_tensor("cpb", [NCHUNK * 128, OUTF], bf16, kind="ExternalInput").ap()
    brow = nc.dram_tensor("brow", [2, OUTF], bf16, kind="ExternalInput").ap()
    gam = nc.dram_tensor("gam", [1, OUTF], f32, kind="ExternalInput").ap()
    bet = nc.dram_tensor("bet", [1, OUTF], f32, kind="ExternalInput").ap()
    out_d = nc.dram_tensor("out", [BC, OUTF], f32, kind="ExternalOutput").ap()

    with tile.TileContext(nc) as tc:
        from contextlib import ExitStack
        with ExitStack() as ctx:
            ep = ctx.enter_context
            consts = ep(tc.tile_pool(name="consts", bufs=1))
            xpool = ep(tc.tile_pool(name="xp", bufs=3))
            spool = ep(tc.tile_pool(name="sp", bufs=12))
            s2pool = ep(tc.tile_pool(name="s2p", bufs=12))
            # the general gamma/beta variant needs 8KB/partition for the
            # broadcast tiles; shave the deep pools to make room
            slim = 1 if (apply_gamma or apply_beta) else 0
            s3pool = ep(tc.tile_pool(name="s3p", bufs=18 - slim))
            prpool = ep(tc.tile_pool(name="prp", bufs=7))
            capool = ep(tc.tile_pool(name="cap", bufs=2))
            colpool = ep(tc.tile_pool(name="colp", bufs=26 - 2 * slim))
            wpool = ep(tc.tile_pool(name="wp", bufs=14 - 2 * slim))
            ypool = ep(tc.tile_pool(name="yp", bufs=5))
            sqpool = ep(tc.tile_pool(name="sqp", bufs=2))
            stpool = ep(tc.tile_pool(name="stp", bufs=9))
            ppool = ep(tc.tile_pool(name="pp", bufs=8, space="PSUM"))

            # --- constants (emitted first; x/weight DMAs overtake via queue) ---
            mconst = consts.tile([128, 16], f32)
            nc.vector.memset(mconst[:, 0:1], EPS)
            # warmup ASAP: the ACT function-table load (~1.3us) overlaps the
            # first x-tile DMA instead of delaying the first basis column
            nc.scalar.copy(mconst[:, 12:13], mconst[:, 0:1])
            for m in range(11):
                nc.vector.memset(mconst[:, m + 1:m + 2], -float(m))
            ones_t = consts.tile([2, 128], bf16)
            nc.vector.memset(ones_t, 1.0)
            brow_t = consts.tile([2, OUTF], bf16)
            gamma_t = (consts.tile([128, OUTF], f32, name="gamma_t")
                       if apply_gamma else None)
            beta_t = (consts.tile([128, OUTF], f32, name="beta_t")
                      if apply_beta else None)

            def emit_const_dmas():
                # brow is first read at macro 0's tail; gamma/beta are only
                # needed from the first macro's flush
                if apply_gamma:
                    nc.sync.dma_start(out=gamma_t,
                                      in_=gam.partition_broadcast(128))
                if apply_beta:
                    nc.sync.dma_start(out=beta_t,
                                      in_=bet.partition_broadcast(128))

            stages = [(bm, ib) for bm in range(NBM) for ib in range(IB)]
            basis_cols = {}    # stage index -> list of 12 col tiles (bf16)
            psums = None       # current macro's psum tiles [bs][oh]
            pending_stats = []  # (bm, bs, y, stt) awaiting sum-of-squares
            pending_math = []  # (bm, bs, y, stt) awaiting mean/var math
            pending_norm = []  # (row, y, stt) awaiting normalize+gamma/beta
            pending_dma = []   # (row, y) awaiting output DMA

            def emit_basis(t, evac_bm=None):
                bm, ib = stages[t]
                xt = xpool.tile([128, BMS], f32, name="xt", tag="xt")
                nc.sync.dma_start(
                    out=xt, in_=xT[ib * 128:(ib + 1) * 128,
                                   bm * BMS:(bm + 1) * BMS])
                cols = [colpool.tile([128, BMS], bf16, name="col", tag="col")
                        for _ in range(12)]
                # immediate columns: linear-x and step(sign)
                nc.scalar.copy(cols[11], xt)
                nc.scalar.activation(out=cols[10], in_=xt, func=AF.Sign,
                                     bias=mconst[:, 11:12], scale=11.0)
                if evac_bm is not None:
                    # previous macro's PSUM evacuation, interleaved here so
                    # it lands right after the two cheap ACT columns instead
                    # of behind the whole basis stage in the engine queues
                    emit_evac(evac_bm)
                s = [None] * 11
                s2 = [None] * 11
                s3 = [None] * 11

                def relu(m):
                    s[m] = spool.tile([128, BMS], f32, name="sm", tag="sm")
                    nc.scalar.activation(out=s[m], in_=xt, func=AF.Relu,
                                         bias=mconst[:, m + 1:m + 2],
                                         scale=11.0)

                def sq(m):
                    s2[m] = s2pool.tile([128, BMS], f32, name="s2m", tag="s2m")
                    nc.scalar.activation(out=s2[m], in_=s[m], func=AF.Square)

                relu(9)
                relu(10)
                # col9 = s9 - 2*s10. DVE owns every fused scalar-tensor-
                # tensor op: the real compiler rejects TensorScalarPtr (and
                # any PSUM access) on Pool, so Pool gets plain tensor-tensor
                # ops only (the s3 muls + 6 pair-adds).
                nc.vector.scalar_tensor_tensor(
                    out=cols[9], in0=s[10], scalar=-2.0, in1=s[9],
                    op0=OP.mult, op1=OP.add)

                pair2 = [None] * 9  # pair2[m] = s3[m] + s3[m+2], m=1..8
                pair4 = [None] * 8  # pair4[k] = s3[k] + s3[k+4], k=0..7

                def cubic(k):
                    # colA = 6*s3[k+2] + pair4[k]; col = -4*pair2[k+1] + colA
                    ca = capool.tile([128, BMS], f32, name="ca", tag="ca")
                    nc.vector.scalar_tensor_tensor(
                        out=ca, in0=s3[k + 2], scalar=6.0, in1=pair4[k],
                        op0=OP.mult, op1=OP.add)
                    nc.vector.scalar_tensor_tensor(
                        out=cols[k], in0=pair2[k + 1], scalar=-4.0, in1=ca,
                        op0=OP.mult, op1=OP.add)

                for m in range(11):
                    if m < 9:
                        relu(m)
                    sq(m)  # squares just-in-time: col8's inputs (sq8..10)
                    # arrive last, so chunk 8 is consumed last (CORDER)
                    # muls live on Pool, which runs a stage ahead (s3pool is
                    # deep); pairs/cols consume s3 on DVE with that slack
                    s3[m] = s3pool.tile([128, BMS], f32, name="s3m", tag="s3m")
                    nc.gpsimd.tensor_mul(s3[m], s2[m], s[m])
                    if m == 7:
                        pair4[7] = s3[7]  # s3[11] == 0 on x in [0,1)
                    if m >= 3:
                        pair2[m - 2] = prpool.tile([128, BMS], f32,
                                                   name="p2", tag="p2")
                        peng = nc.vector if m <= 5 else nc.gpsimd
                        peng.tensor_add(pair2[m - 2], s3[m - 2], s3[m])
                    if m >= 4:
                        pair4[m - 4] = prpool.tile([128, BMS], f32,
                                                   name="p4", tag="p4")
                        nc.vector.tensor_add(pair4[m - 4], s3[m - 4], s3[m])
                        cubic(m - 4)
                # tail: pair4[7] aliases s3[7]; pair2[8] ready at m=10
                cubic(7)
                # col8 = (s8^2 - 3 s9^2) + 3 s10^2, emitted last
                qa = capool.tile([128, BMS], f32, name="qa", tag="qa")
                nc.vector.scalar_tensor_tensor(
                    out=qa, in0=s2[9], scalar=-3.0, in1=s2[8],
                    op0=OP.mult, op1=OP.add)
                nc.vector.scalar_tensor_tensor(
                    out=cols[8], in0=s2[10], scalar=3.0, in1=qa,
                    op0=OP.mult, op1=OP.add)
                basis_cols[t] = cols

            def emit_matmuls(t):
                nonlocal psums
                bm, ib = stages[t]
                bias_leads = (ib == 0 and bm > 0)
                if ib == 0:
                    psums = [[ppool.tile([128, 512], f32, name="psum",
                                         tag="psum") for _ in range(2)]
                             for _ in range(4)]
                if bias_leads:
                    # bias row leads the macro: it has no basis dependency,
                    # so the PE restarts the moment each PSUM pair frees
                    for bs_i in range(4):
                        for oh in range(2):
                            nc.tensor.matmul(
                                psums[bs_i][oh], ones_t,
                                brow_t[:, oh * 512:(oh + 1) * 512],
                                start=True, stop=False)
                cols = basis_cols.pop(t)

                def mm(c, wt, bs_range=range(4), stop=False):
                    for bs_i in bs_range:
                        first = (not bias_leads and ib == 0
                                 and c == CORDER[0])
                        lhsT = cols[c][:, bs_i * 128:(bs_i + 1) * 128]
                        for oh in range(2):
                            nc.tensor.matmul(
                                psums[bs_i][oh], lhsT,
                                wt[:, oh * 512:(oh + 1) * 512],
                                start=first, stop=stop)

                wts = {}
                for c in CORDER:
                    chunk = ib * CPI + c
                    wt = wpool.tile([128, OUTF], bf16, name="wt", tag="wt")
                    nc.sync.dma_start(
                        out=wt, in_=cpb[chunk * 128:(chunk + 1) * 128, :])
                    wts[c] = wt
                if ib == IB - 1:
                    # bs-major order for the last 3 chunks: each psum's
                    # accumulation finishes early, so its evacuation
                    # overlaps the remaining batch-blocks' matmuls. The
                    # final macro runs reversed (bs3 first) so the epilogue
                    # chain of every block starts as early as possible.
                    for c in CORDER[:-3]:
                        mm(c, wts[c])
                    for bs_i in range(4):
                        bias_tail = (bm == 0)
                        for c in CORDER[-3:]:
                            mm(c, wts[c], bs_range=(bs_i,),
                               stop=(not bias_tail and c == CORDER[-1]))
                        if bias_tail:
                            for oh in range(2):
                                nc.tensor.matmul(
                                    psums[bs_i][oh], ones_t,
                                    brow_t[:, oh * 512:(oh + 1) * 512],
                                    start=False, stop=True)
                else:
                    for c in CORDER:
                        mm(c, wts[c])

            def emit_evac(bm):
                # stt cols: 0 sum_a, 1 sum_b, 2 sq_a, 3 sq_b, 4 S, 5 Q,
                #           6 mean, 7 E[z^2], 8 mean^2, 9 var, 10 sig, 11 rstd
                # Evacuate PSUM with ACT copies (+ DVE for 2 blocks): these
                # gate the next macro's matmuls, so they get wait-ts 0 and
                # high priority — the pacing on basis stages (see emit loop)
                # keeps the scheduler from filling the boundary slots with
                # future basis work first.
                last = (bm == NBM - 1)
                with tc.tile_wait_until(0.0), tc.high_priority():
                    for bs_i in range(4):
                        stt = stpool.tile([128, 12], f32, name="stt",
                                          tag="stt")
                        y = ypool.tile([128, OUTF], f32, name="y", tag="y")
                        for oh in range(2):
                            if bs_i in (1, 2):
                                nc.vector.tensor_scalar(
                                    out=y[:, oh * 512:(oh + 1) * 512],
                                    in0=psums[bs_i][oh], scalar1=1.0,
                                    scalar2=0.0, op0=OP.mult, op1=OP.add,
                                    accum_out=stt[:, oh:oh + 1])
                            else:
                                nc.scalar.activation(
                                    out=y[:, oh * 512:(oh + 1) * 512],
                                    in_=psums[bs_i][oh], func=AF.Copy,
                                    accum_out=stt[:, oh:oh + 1])
                        pending_stats.append((bm, bs_i, y, stt))

            def squares_flush(last=False):
                # row sums + sums-of-squares off the y staging buffer,
                # deferred one stage past the evacuation (off PE's path)
                items = list(pending_stats)
                pending_stats.clear()
                for bm, bs_i, y, stt in items:
                    for oh in range(2):
                        yh = y[:, oh * 512:(oh + 1) * 512]
                        sqs = sqpool.tile([128, 512], f32, name="sqs",
                                          tag="sqs")
                        if last and bs_i in (1, 3):
                            nc.vector.scalar_tensor_tensor(
                                out=sqs, in0=yh, scalar=1.0, in1=yh,
                                op0=OP.mult, op1=OP.mult,
                                accum_out=stt[:, 2 + oh:3 + oh])
                        else:
                            nc.scalar.activation(
                                out=sqs, in_=yh, func=AF.Square,
                                accum_out=stt[:, 2 + oh:3 + oh])
                    pending_math.append((bm, bs_i, y, stt))

            def statsmath_flush():
                # per-row scalar math, a further stage later so the DVE ops
                # never head-of-line block behind the ACT squares
                items = list(pending_math)
                pending_math.clear()
                for bm, bs_i, y, stt in items:
                    nc.vector.tensor_add(stt[:, 4:5], stt[:, 0:1], stt[:, 1:2])
                    nc.vector.tensor_add(stt[:, 5:6], stt[:, 2:3], stt[:, 3:4])
                    nc.vector.tensor_scalar_mul(stt[:, 6:7], stt[:, 4:5],
                                                1.0 / OUTF)
                    nc.vector.tensor_scalar_mul(stt[:, 7:8], stt[:, 5:6],
                                                1.0 / OUTF)
                    nc.vector.scalar_tensor_tensor(
                        out=stt[:, 8:9], in0=stt[:, 6:7], scalar=1.0,
                        in1=stt[:, 6:7], op0=OP.mult, op1=OP.mult)
                    nc.vector.scalar_tensor_tensor(
                        out=stt[:, 9:10], in0=stt[:, 8:9], scalar=-1.0,
                        in1=stt[:, 7:8], op0=OP.mult, op1=OP.add)
                    nc.scalar.activation(out=stt[:, 10:11], in_=stt[:, 9:10],
                                         func=AF.Sqrt, bias=mconst[:, 0:1])
                    nc.vector.reciprocal(out=stt[:, 11:12], in_=stt[:, 10:11])
                    pending_norm.append((bm * BMS + bs_i * 128, y, stt))

            def norm_flush(limit=None):
                # normalize + gamma/beta, deferred so the big [128,1024] ops
                # stay clear of the macro boundary
                idx = 0
                n = limit if limit else len(pending_norm)
                while pending_norm and n > 0:
                    n -= 1
                    row, y, stt = pending_norm.pop(0)
                    # tensor_scalar must be DVE (Pool lacks TensorScalarPtr)
                    nc.vector.tensor_scalar(out=y, in0=y, scalar1=stt[:, 6:7],
                                            scalar2=stt[:, 11:12],
                                            op0=OP.subtract, op1=OP.mult)
                    eng = nc.gpsimd if idx % 2 == 0 else nc.vector
                    if apply_gamma:
                        eng.tensor_mul(y, y, gamma_t)
                    if apply_beta:
                        eng.tensor_add(y, y, beta_t)
                    pending_dma.append((row, y))
                    idx += 1

            def dma_flush(limit=None, dual_queue=False):
                idx = 0
                n = limit if limit else len(pending_dma)
                while pending_dma and n > 0:
                    n -= 1
                    row, y = pending_dma.pop(0)
                    eng = nc.scalar if dual_queue and idx % 2 else nc.sync
                    eng.dma_start(out=out_d[row:row + 128, :], in_=y)
                    idx += 1

            # Manual pacing: cap the scheduler's lookahead for each basis
            # stage at ~1.3 stages ahead of the PE so macro-boundary slots
            # stay open for the (wait-ts 0) PSUM evacuation ops. This only
            # constrains the compile-time instruction ORDER; runtime is
            # still fully semaphore-driven.
            STAGE_MS = 0.0207

            emit_basis(0)
            emit_matmuls(0)
            nc.scalar.dma_start(out=brow_t, in_=brow)  # ACT queue: off SP's path
            emit_const_dmas()
            for t, (bm, ib) in enumerate(stages):
                if t == 0:
                    continue
                with tc.tile_wait_until(max(t * 0.002, (t - 1.3) * STAGE_MS)):
                    emit_basis(t, evac_bm=bm - 1 if ib == 0 else None)
                    if ib == 1:
                        squares_flush()
                    if ib == 3:
                        statsmath_flush()
                    if ib == 5:
                        norm_flush()
                    if ib == 6:
                        dma_flush()
                emit_matmuls(t)

            # final macro: normalize straight out of PSUM (no staging copy,
            # no deferral — nothing left to overlap with). bn_stats feeds an
            # ACT Identity with per-row scale/bias; DMAs ride both queues.
            bm = NBM - 1
            with tc.tile_wait_until(0.0), tc.high_priority():
                for bs_i in range(4):
                    stt = stpool.tile([128, 20], f32, name="fstt", tag="fstt")
                    stats = stt[:, 0:12].rearrange("p (g s) -> p g s", g=2)
                    mvsi = stt[:, 12:16]
                    nc.vector.bn_stats(out=stats[:, 0, :], in_=psums[bs_i][0])
                    nc.vector.bn_stats(out=stats[:, 1, :], in_=psums[bs_i][1])
                    nc.vector.bn_aggr(out=mvsi[:, 0:2], in_=stats)
                    nc.scalar.activation(out=mvsi[:, 2:3], in_=mvsi[:, 1:2],
                                         func=AF.Sqrt, bias=mconst[:, 0:1])
                    nc.vector.reciprocal(out=mvsi[:, 3:4], in_=mvsi[:, 2:3])
                    nc.vector.scalar_tensor_tensor(
                        out=stt[:, 16:17], in0=mvsi[:, 0:1], scalar=-1.0,
                        in1=mvsi[:, 3:4], op0=OP.mult, op1=OP.mult)
                    y = ypool.tile([128, OUTF], f32, name="y", tag="y")
                    row = bm * BMS + bs_i * 128
                    eng = nc.sync  # single queue: ACT-queue DMAs would HOL-block its compute
                    for oh in range(2):
                        if bs_i in (1, 2, 3):
                            nc.vector.tensor_scalar(
                                out=y[:, oh * 512:(oh + 1) * 512],
                                in0=psums[bs_i][oh], scalar1=mvsi[:, 0:1],
                                scalar2=mvsi[:, 3:4],
                                op0=OP.subtract, op1=OP.mult)
                        else:
                            nc.scalar.activation(
                                out=y[:, oh * 512:(oh + 1) * 512],
                                in_=psums[bs_i][oh], func=AF.Identity,
                                bias=stt[:, 16:17], scale=mvsi[:, 3:4])
                        if not (apply_gamma or apply_beta):
                            # half-granularity DMA: ship each half while the
                            # other is still normalizing
                            eng.dma_start(
                                out=out_d[row:row + 128,
                                          oh * 512:(oh + 1) * 512],
                                in_=y[:, oh * 512:(oh + 1) * 512])
                    if apply_gamma or apply_beta:
                        if apply_gamma:
                            nc.vector.tensor_mul(y, y, gamma_t)
                        if apply_beta:
                            nc.gpsimd.tensor_add(y, y, beta_t)
                        eng.dma_start(out=out_d[row:row + 128, :], in_=y)

    nc.compile()
    return nc


def _host_prep(x, control_points, W, b):
    """Build per-core inputs. cpb row (ib*12+k)*128+r holds the weights for
    feature (k, i=ib*128+r); scale factors folded in."""
    cp64 = control_points.astype(np.float64)
    blocks = []
    for k in range(12):
        if k < 8:
            blk = cp64[:, :, k].T / 6.0
        elif k == 8:
            blk = cp64[:, :, 8].T / 2.0
        elif k == 9:
            blk = cp64[:, :, 9].T
        elif k == 10:
            blk = cp64[:, :, 10].T / 2.0
        else:
            blk = W.astype(np.float64).T
        blocks.append(blk)
    # device chunk order: chunk = ib*12 + k  (i-block major, feature minor)
    kmaj = np.concatenate(blocks, axis=0).reshape(12, IB, 128, OUTF)
    cpb = np.ascontiguousarray(
        kmaj.transpose(1, 0, 2, 3).reshape(12 * INF, OUTF)
    ).astype(ml_dtypes.bfloat16)
    brow_f64 = b.astype(np.float64) + 0.5 * cp64[:, :, 10].sum(axis=1)
    brow_hi = brow_f64.astype(ml_dtypes.bfloat16)
    brow_lo = (brow_f64 - brow_hi.astype(np.float64)).astype(ml_dtypes.bfloat16)
    brow = np.ascontiguousarray(np.stack([brow_hi, brow_lo], axis=0))
    xT = np.ascontiguousarray(x.T)  # [INF, BATCH]
    return xT, cpb, brow


_last_flags = (True, True)


def kernel(x, control_points, W, b, gamma, beta):
    global _last_flags
    from concourse.bass_utils import run_bass_kernel_spmd

    xT, cpb, brow = _host_prep(x, control_points, W, b)
    gam = np.ascontiguousarray(gamma.astype(np.float32))[None, :]
    bet = np.ascontiguousarray(beta.astype(np.float32))[None, :]

    # multiplying by an all-ones gamma / adding an all-zeros beta is the
    # identity; build a variant without those ops when the inputs allow
    _last_flags = (bool(np.any(gamma != 1.0)), bool(np.any(beta != 0.0)))
    nc = _build_nc(*_last_flags)
    in_maps = []
    for c in range(NCORES):
        in_maps.append({
            "xT": np.ascontiguousarray(xT[:, c * BC:(c + 1) * BC]),
            "cpb": cpb,
            "brow": brow,
            "gam": gam,
            "bet": bet,
        })
    res = run_bass_kernel_spmd(nc, in_maps, list(range(NCORES)))
    out = np.concatenate([res.results[c]["out"] for c in range(NCORES)], axis=0)
    return out
